# revision 11
# baseline (speedup 1.0000x reference)
import sys
import time

sys.path.insert(0, "/opt/trn_rl_repo")

import numpy as np
import ml_dtypes

import jax

# Persistent compilation cache: repeat dispatches of the same program skip
# the backend (walrus/NEFF) compile entirely.
try:
    jax.config.update("jax_compilation_cache_dir", "/tmp/jax_ccache")
    jax.config.update("jax_persistent_cache_min_entry_size_bytes", -1)
    jax.config.update("jax_persistent_cache_min_compile_time_secs", 0.0)
except Exception:
    pass

from concourse import bass, mybir, tile, bacc, bass_utils
from concourse.bass import ds

BF16 = mybir.dt.bfloat16
F32 = mybir.dt.float32
AF = mybir.ActivationFunctionType

T, B, F, H, L = 1024, 256, 128, 512, 256
NCORES = 8
BL = B // NCORES  # 32 batch rows per core
U = 16            # steps per hw-loop body == ring depth
NB = T // U       # 64 blocks

# gate strip order within a 512-wide strip: i, f, o, g (each 128 cols)
# source gate row offsets in the 4H weight rows (pytorch order i,f,g,o):
GOFF = (0 * H, 1 * H, 3 * H, 2 * H)  # i, f, o, g


def _bf(x):
    return np.ascontiguousarray(x).astype(ml_dtypes.bfloat16)


def _perm_cols(w4h_by_k):
    """Build out[s, j, :] = w4h_by_k[GOFF[j//128] + 128*s + j%128, :]."""
    out = np.empty((4, 512) + w4h_by_k.shape[1:], w4h_by_k.dtype)
    for s in range(4):
        for jg in range(4):
            rows = GOFF[jg] + 128 * s + np.arange(128)
            out[s, jg * 128 : (jg + 1) * 128] = w4h_by_k[rows]
    return out


def pack_rec(Whh):
    """Whh: [2048, 512] -> [128, 4k, 4s, 512] : arr[p,k,s,j] = Whh[gr(s,j), 128k+p]"""
    perm = _perm_cols(Whh)  # [4s, 512j, 512K]
    arr = perm.transpose(2, 0, 1).reshape(4, 128, 4, 512).transpose(1, 0, 2, 3)
    return _bf(arr)  # [128p, 4k, 4s, 512j]


def pack_x(Wih, k0, nk):
    """Wih: [2048, Kx] cols [k0*128 : k0*128+nk*128] -> [128, nk, 4s, 512]"""
    perm = _perm_cols(Wih[:, k0 * 128 : k0 * 128 + nk * 128])  # [4,512,nk*128]
    arr = perm.transpose(2, 0, 1).reshape(nk, 128, 4, 512).transpose(1, 0, 2, 3)
    return _bf(arr)


def pack_rows(mat):
    """mat: [nr, 2048] -> [nr, 4s, 512] with strip permutation."""
    perm = _perm_cols(mat.T)  # [4, 512, nr]
    return _bf(perm.transpose(2, 0, 1))


_CACHE = {}


def build(Tn, nb=None, repeat=1):
    assert Tn == T
    if nb is None:
        nb = NB
    nc = bacc.Bacc("TRN2", target_bir_lowering=False, debug=False,
                   num_devices=NCORES)
    d = {}

    def din(name, shape, dt=BF16):
        d[name] = nc.dram_tensor(name, shape, dt, kind="ExternalInput")
        return d[name]

    # streamed inputs, partition-major [P, T, ...]
    din("xcT", (128, Tn, BL))          # constraints.T, time-reversed
    din("xce", (2, Tn, BL))            # row0: 129th input, row1: ones
    din("xsT", (128, Tn, BL))          # seq.T (for g0, used at t-1)
    # weights
    din("w_rec_c0", (128, 4, 4, 512)); din("w_rec_c1", (128, 4, 4, 512))
    din("w_rec_g0", (128, 4, 4, 512)); din("w_rec_g1", (128, 4, 4, 512))
    din("w_x_c0", (128, 1, 4, 512))    # k-tile 0 of c0_Wih
    din("w_xe_c0", (2, 4, 512))        # [row129 ; bias_c0]
    din("w_x_c1", (128, 4, 4, 512)); din("w_b_c1", (1, 4, 512))
    din("w_xs_g0", (128, 1, 4, 512))   # seq part of g0_Wih
    din("w_xc_g0", (128, 4, 4, 512)); din("w_b_g0", (1, 4, 512))
    din("w_x_g1", (128, 4, 4, 512)); din("w_b_g1", (1, 4, 512))
    din("w_l1", (128, 4, 256)); din("w_bl1", (1, 256))
    din("w_l2", (128, 2, 128)); din("w_bl2", (1, 128))
    din("id32", (128, 32))             # stacked I32 blocks
    din("id128", (128, 128))           # I128
    din("ones1", (1, 128))             # ones row (bias matmuls)
    # spill + output
    c1hT_d = nc.dram_tensor("c1hT", (128, Tn, 128), BF16, kind="Internal")
    out_d = nc.dram_tensor("preds", (128, Tn // 4, 128), BF16,
                           kind="ExternalOutput")

    with tile.TileContext(nc) as tc:
        with (
            tc.tile_pool(name="wpool", bufs=1) as wp,
            tc.tile_pool(name="ring", bufs=1) as rp,
            tc.tile_pool(name="stream", bufs=2) as sp,
            tc.tile_pool(name="ew", bufs=3) as ep,
            tc.tile_pool(name="gates_ps", bufs=4, space="PSUM") as gp,
            tc.tile_pool(name="ht_ps", bufs=2, space="PSUM") as hp,
            tc.tile_pool(name="mlp_ps", bufs=2, space="PSUM") as mp,
        ):
            # ---- load weights / constants into SBUF (resident) ----
            W = {}
            for nm in ("w_rec_c0", "w_rec_c1", "w_rec_g0", "w_rec_g1",
                       "w_x_c1", "w_xc_g0", "w_x_g1"):
                W[nm] = wp.tile([128, 4, 4, 512], BF16, name=nm + "_sb")
                nc.sync.dma_start(W[nm][:], d[nm].ap())
            for nm, shp in (("w_x_c0", [128, 1, 4, 512]),
                            ("w_xs_g0", [128, 1, 4, 512]),
                            ("w_xe_c0", [2, 4, 512]),
                            ("w_b_c1", [1, 4, 512]), ("w_b_g0", [1, 4, 512]),
                            ("w_b_g1", [1, 4, 512]),
                            ("w_l1", [128, 4, 256]), ("w_bl1", [1, 256]),
                            ("w_l2", [128, 2, 128]), ("w_bl2", [1, 128]),
                            ("id32", [128, 32]), ("id128", [128, 128]),
                            ("ones1", [1, 128])):
                W[nm] = wp.tile(shp, BF16, name=nm + "_sb")
                nc.sync.dma_start(W[nm][:], d[nm].ap())

            # ---- persistent state ----
            hTr = {}
            for l in ("c0", "c1", "g0", "g1"):
                hTr[l] = rp.tile([128, U, 128], BF16, name=f"hTr_{l}")
            cst = {l: rp.tile([128, 128], F32, name=f"c_{l}")
                   for l in ("c0", "c1", "g0", "g1")}

            def lstm_mms(l, slot_prev, x_mms, wrec):
                """Gates matmuls; slot_prev = ring slot of h_{s-1} (None at
                the very first step). Returns the gates psum tile."""
                gates = gp.tile([128, 512], F32, name="gates", tag="gates")
                mms = list(x_mms)
                if slot_prev is not None:
                    for k in range(4):
                        mms.append((hTr[l][:, slot_prev, 32 * k:32 * k + 32],
                                    wrec[:, k]))
                for st in range(4):
                    for i, (lhsT, rhs) in enumerate(mms):
                        nc.tensor.matmul(
                            gates[32 * st:32 * st + 32, :], lhsT,
                            rhs[:, st, :],
                            start=(i == 0), stop=(i == len(mms) - 1),
                            tile_position=(0, 32 * st),
                        )
                return gates

            def lstm_tail(l, slot_out, gates, first=False):
                sig = ep.tile([128, 384], F32, name="sig", tag="sig")
                nc.scalar.activation(sig[:], gates[:, 0:384], AF.Sigmoid)
                tg = ep.tile([128, 128], F32, name="tg", tag="tg")
                nc.scalar.activation(tg[:], gates[:, 384:512], AF.Tanh)
                ig = ep.tile([128, 128], F32, name="ig", tag="ig")
                nc.vector.tensor_mul(ig[:], sig[:, 0:128], tg[:])
                c = cst[l]
                if not first:
                    fc = ep.tile([128, 128], F32, name="fc", tag="fc")
                    nc.vector.tensor_mul(fc[:], sig[:, 128:256], c[:])
                    nc.vector.tensor_add(c[:], ig[:], fc[:])
                else:
                    nc.vector.tensor_copy(c[:], ig[:])
                tc_ = ep.tile([128, 128], F32, name="tc_", tag="tc_")
                nc.scalar.activation(tc_[:], c[:], AF.Tanh)
                h = ep.tile([128, 128], BF16, name="h", tag="h")
                nc.vector.tensor_mul(h[:], sig[:, 256:384], tc_[:])
                # transpose h -> hT ring (full 128x128: out block q = h_q.T)
                hps = hp.tile([128, 128], BF16, name="hps", tag="hps")
                nc.tensor.transpose(hps[:], h[:], W["id128"][:])
                nc.vector.tensor_copy(hTr[l][:, slot_out, :], hps[:])

            def c_iteration(j, xc_ch, xce_ch, first_block):
                """One interleaved scan position of phase C: c0 step at ring
                slot j, c1 step at slot (j-1)%U. first_block: python bool,
                j==0 of the peeled block 0."""
                fs = first_block and j == 0
                x_mms = [
                    (xc_ch[:, j, :], W["w_x_c0"][:, 0]),
                    (xce_ch[:, j, :], W["w_xe_c0"]),
                ]
                g_c0 = lstm_mms("c0", None if fs else (j - 1) % U, x_mms,
                                W["w_rec_c0"])
                g_c1 = None
                if not fs:
                    sl = (j - 1) % U
                    fs1 = first_block and j == 1
                    x_mms = [(hTr["c0"][:, sl, 32 * k:32 * k + 32],
                              W["w_x_c1"][:, k]) for k in range(4)]
                    x_mms.append((W["ones1"][:, 0:BL], W["w_b_c1"]))
                    g_c1 = lstm_mms("c1", None if fs1 else (j - 2) % U,
                                    x_mms, W["w_rec_c1"])
                lstm_tail("c0", j, g_c0, first=fs)
                if g_c1 is not None:
                    lstm_tail("c1", (j - 1) % U, g_c1,
                              first=(first_block and j == 1))

            def mlp(slots, out_off):
                """MLP head on g1 ring slots (len 4) -> preds block out_off."""
                mo = mp.tile([128, 256], F32, name="mo", tag="mlp")
                for tau in range(4):
                    for k in range(4):
                        nc.tensor.matmul(
                            mo[32 * tau:32 * tau + 32, :],
                            hTr["g1"][:, slots[tau], 32 * k:32 * k + 32],
                            W["w_l1"][:, k, :], start=(k == 0),
                            stop=False, tile_position=(0, 32 * tau))
                    nc.tensor.matmul(
                        mo[32 * tau:32 * tau + 32, :],
                        W["ones1"][:, 0:32], W["w_bl1"][:],
                        start=False, stop=True,
                        tile_position=(0, 32 * tau))
                h1 = ep.tile([128, 256], BF16, name="h1", tag="h1")
                nc.scalar.activation(h1[:], mo[:], AF.Relu)
                h1t = mp.tile([128, 256], BF16, name="h1t", tag="mlp")
                for j in range(2):
                    nc.tensor.transpose(
                        h1t[:, 128 * j:128 * j + 128],
                        h1[:, 128 * j:128 * j + 128], W["id128"][:])
                h1ts = ep.tile([128, 256], BF16, name="h1ts", tag="h1ts")
                nc.vector.tensor_copy(h1ts[:], h1t[:])
                po = mp.tile([128, 128], F32, name="po", tag="mlp")
                for k in range(2):
                    nc.tensor.matmul(
                        po[:], h1ts[:, 128 * k:128 * k + 128],
                        W["w_l2"][:, k, :], start=(k == 0), stop=False)
                nc.tensor.matmul(po[:], W["ones1"][:], W["w_bl2"][:],
                                 start=False, stop=True)
                ps = ep.tile([128, 128], BF16, name="ps", tag="ps")
                nc.vector.tensor_copy(ps[:], po[:])
                nc.sync.dma_start(out_d.ap()[:, ds(out_off, 1), :],
                                  ps[:, None, :])

            def g_iteration(j, xs_ch, cg_ch, first_block):
                """One interleaved scan position of phase G: g0 step at ring
                slot j (inputs: cg slot 15-j, xs slot j), g1 at (j-1)%U."""
                fs = first_block and j == 0
                x_mms = [(cg_ch[:, U - 1 - j, 32 * k:32 * k + 32],
                          W["w_xc_g0"][:, k]) for k in range(4)]
                if not fs:
                    x_mms.append((xs_ch[:, j, :], W["w_xs_g0"][:, 0]))
                x_mms.append((W["ones1"][:, 0:BL], W["w_b_g0"]))
                g_g0 = lstm_mms("g0", None if fs else (j - 1) % U, x_mms,
                                W["w_rec_g0"])
                g_g1 = None
                if not fs:
                    sl = (j - 1) % U
                    fs1 = first_block and j == 1
                    x_mms = [(hTr["g0"][:, sl, 32 * k:32 * k + 32],
                              W["w_x_g1"][:, k]) for k in range(4)]
                    x_mms.append((W["ones1"][:, 0:BL], W["w_b_g1"]))
                    g_g1 = lstm_mms("g1", None if fs1 else (j - 2) % U,
                                    x_mms, W["w_rec_g1"])
                lstm_tail("g0", j, g_g0, first=fs)
                if g_g1 is not None:
                    lstm_tail("g1", (j - 1) % U, g_g1,
                              first=(first_block and j == 1))

            def run_all():
                # ================= phase C =================
                # --- peeled block 0 (steps 0..15) ---
                xc0 = sp.tile([128, U, BL], BF16, name="xc0", tag="xc")
                nc.sync.dma_start(xc0[:], d["xcT"].ap()[:, 0:U, :])
                xce0 = sp.tile([2, U, BL], BF16, name="xce0", tag="xce")
                nc.sync.dma_start(xce0[:], d["xce"].ap()[:, 0:U, :])
                for j in range(U):
                    c_iteration(j, xc0, xce0, first_block=True)
                    if j == 8:
                        # c1 steps [0,8) complete (slots 0..7)
                        nc.sync.dma_start(c1hT_d.ap()[:, 0:8, :],
                                          hTr["c1"][:, 0:8, :])

                # --- blocks 1..63 ---
                with tc.For_i(1, nb, name="loopC") as bi:
                    xc_ch = sp.tile([128, U, BL], BF16, name="xc_ch", tag="xc")
                    nc.sync.dma_start(xc_ch[:],
                                      d["xcT"].ap()[:, ds(bi * U, U), :])
                    xce_ch = sp.tile([2, U, BL], BF16, name="xce_ch",
                                     tag="xce")
                    nc.sync.dma_start(xce_ch[:],
                                      d["xce"].ap()[:, ds(bi * U, U), :])
                    for j in range(U):
                        c_iteration(j, xc_ch, xce_ch, first_block=False)
                        if j == 0:
                            # c1 steps [16bi-8, 16bi) done (slots 8..15)
                            nc.sync.dma_start(
                                c1hT_d.ap()[:, ds(bi * U - 8, 8), :],
                                hTr["c1"][:, 8:U, :])
                        elif j == 8:
                            # c1 steps [16bi, 16bi+8) done (slots 0..7)
                            nc.sync.dma_start(
                                c1hT_d.ap()[:, ds(bi * U, 8), :],
                                hTr["c1"][:, 0:8, :])

                # --- epilogue: c1 step 1023 ---
                x_mms = [(hTr["c0"][:, U - 1, 32 * k:32 * k + 32],
                          W["w_x_c1"][:, k]) for k in range(4)]
                x_mms.append((W["ones1"][:, 0:BL], W["w_b_c1"]))
                g_c1 = lstm_mms("c1", U - 2, x_mms, W["w_rec_c1"])
                lstm_tail("c1", U - 1, g_c1)
                nc.sync.dma_start(c1hT_d.ap()[:, Tn - 8:Tn, :],
                                  hTr["c1"][:, 8:U, :])

                # ================= phase G =================
                # --- peeled block 0 (steps 0..15) ---
                xs0 = sp.tile([128, U, BL], BF16, name="xs0", tag="xs")
                nc.sync.dma_start(xs0[:, 1:U, :], d["xsT"].ap()[:, 0:U - 1, :])
                cg0 = sp.tile([128, U, 128], BF16, name="cg0", tag="cg")
                nc.sync.dma_start(cg0[:], c1hT_d.ap()[:, Tn - U:Tn, :])
                for j in range(U):
                    g_iteration(j, xs0, cg0, first_block=True)
                    if j in (4, 8, 12):
                        # g1 steps [j-4, j) complete (slots j-4..j-1)
                        mlp(list(range(j - 4, j)), j // 4 - 1)

                # --- blocks 1..63 ---
                with tc.For_i(1, nb, name="loopG") as bi:
                    xs_ch = sp.tile([128, U, BL], BF16, name="xs_ch", tag="xs")
                    nc.sync.dma_start(xs_ch[:],
                                      d["xsT"].ap()[:, ds(bi * U - 1, U), :])
                    cg_ch = sp.tile([128, U, 128], BF16, name="cg_ch",
                                    tag="cg")
                    nc.sync.dma_start(cg_ch[:],
                                      c1hT_d.ap()[:, ds(Tn - U - bi * U, U), :])
                    for j in range(U):
                        g_iteration(j, xs_ch, cg_ch, first_block=False)
                        if j == 0:
                            mlp([12, 13, 14, 15], bi * 4 - 1)
                        elif j in (4, 8, 12):
                            mlp(list(range(j - 4, j)), bi * 4 + j // 4 - 1)

                # --- epilogue: g1 step 1023 + last MLP block ---
                x_mms = [(hTr["g0"][:, U - 1, 32 * k:32 * k + 32],
                          W["w_x_g1"][:, k]) for k in range(4)]
                x_mms.append((W["ones1"][:, 0:BL], W["w_b_g1"]))
                g_g1 = lstm_mms("g1", U - 2, x_mms, W["w_rec_g1"])
                lstm_tail("g1", U - 1, g_g1)
                mlp([12, 13, 14, 15], Tn // 4 - 1)

            if repeat == 1:
                run_all()
            else:
                with tc.For_i(0, repeat, name="rep"):
                    run_all()

    nc.compile()
    return nc


def prepack(inputs, Tn=T):
    """Returns per-core input dicts (weights shared, acts sliced)."""
    f32 = np.float32
    sc = np.asarray(inputs["seq_constraints"], f32)[:Tn]
    sq = np.asarray(inputs["seq"], f32)[:Tn]
    shared = {}
    shared["w_rec_c0"] = pack_rec(np.asarray(inputs["c0_Whh"], f32))
    shared["w_rec_c1"] = pack_rec(np.asarray(inputs["c1_Whh"], f32))
    shared["w_rec_g0"] = pack_rec(np.asarray(inputs["g0_Whh"], f32))
    shared["w_rec_g1"] = pack_rec(np.asarray(inputs["g1_Whh"], f32))
    c0W = np.asarray(inputs["c0_Wih"], f32)
    shared["w_x_c0"] = pack_x(c0W, 0, 1)
    b_c0 = np.asarray(inputs["c0_bih"], f32) + np.asarray(inputs["c0_bhh"], f32)
    shared["w_xe_c0"] = pack_rows(np.stack([c0W[:, 128], b_c0]))
    shared["w_x_c1"] = pack_x(np.asarray(inputs["c1_Wih"], f32), 0, 4)
    shared["w_b_c1"] = pack_rows(
        (np.asarray(inputs["c1_bih"], f32) + np.asarray(inputs["c1_bhh"], f32))[None])
    g0W = np.asarray(inputs["g0_Wih"], f32)
    shared["w_xs_g0"] = pack_x(g0W, 0, 1)
    shared["w_xc_g0"] = pack_x(g0W, 1, 4)
    shared["w_b_g0"] = pack_rows(
        (np.asarray(inputs["g0_bih"], f32) + np.asarray(inputs["g0_bhh"], f32))[None])
    shared["w_x_g1"] = pack_x(np.asarray(inputs["g1_Wih"], f32), 0, 4)
    shared["w_b_g1"] = pack_rows(
        (np.asarray(inputs["g1_bih"], f32) + np.asarray(inputs["g1_bhh"], f32))[None])
    shared["w_l1"] = _bf(np.asarray(inputs["lin1_W"], f32).T.reshape(4, 128, 256)
                         .transpose(1, 0, 2))
    shared["w_bl1"] = _bf(np.asarray(inputs["lin1_b"], f32)[None])
    shared["w_l2"] = _bf(np.asarray(inputs["lin2_W"], f32).T.reshape(2, 128, 128)
                         .transpose(1, 0, 2))
    shared["w_bl2"] = _bf(np.asarray(inputs["lin2_b"], f32)[None])
    id32 = np.zeros((128, 32), f32)
    for q in range(4):
        id32[32 * q:32 * q + 32] = np.eye(32)
    shared["id32"] = _bf(id32)
    shared["id128"] = _bf(np.eye(128))
    shared["ones1"] = _bf(np.ones((1, 128)))

    in_maps = []
    for c in range(NCORES):
        bs = slice(BL * c, BL * (c + 1))
        m = dict(shared)
        xc_rev = sc[::-1, bs, :]  # [Tn, BL, 129]
        m["xcT"] = _bf(xc_rev[:, :, :128].transpose(2, 0, 1))
        xce = np.empty((2, Tn, BL), f32)
        xce[0] = xc_rev[:, :, 128].reshape(Tn, BL)
        xce[1] = 1.0
        m["xce"] = _bf(xce)
        m["xsT"] = _bf(sq[:, bs, :].transpose(2, 0, 1))
        in_maps.append(m)
    return in_maps


def build_null(Tn):
    """Same I/O footprint as build(), near-empty program (for timing)."""
    nc = bacc.Bacc("TRN2", target_bir_lowering=False, debug=False,
                   num_devices=NCORES)
    names = [("xcT", (128, Tn, BL)), ("xce", (2, Tn, BL)),
             ("xsT", (128, Tn, BL)),
             ("w_rec_c0", (128, 4, 4, 512)), ("w_rec_c1", (128, 4, 4, 512)),
             ("w_rec_g0", (128, 4, 4, 512)), ("w_rec_g1", (128, 4, 4, 512)),
             ("w_x_c0", (128, 1, 4, 512)), ("w_xe_c0", (2, 4, 512)),
             ("w_x_c1", (128, 4, 4, 512)), ("w_b_c1", (1, 4, 512)),
             ("w_xs_g0", (128, 1, 4, 512)), ("w_xc_g0", (128, 4, 4, 512)),
             ("w_b_g0", (1, 4, 512)), ("w_x_g1", (128, 4, 4, 512)),
             ("w_b_g1", (1, 4, 512)), ("w_l1", (128, 4, 256)),
             ("w_bl1", (1, 256)), ("w_l2", (128, 2, 128)),
             ("w_bl2", (1, 128)), ("id32", (128, 32)), ("id128", (128, 128)),
             ("ones1", (1, 128))]
    for nm, shp in names:
        nc.dram_tensor(nm, shp, BF16, kind="ExternalInput")
    out_d = nc.dram_tensor("preds", (128, Tn // 4, 128), BF16,
                           kind="ExternalOutput")
    with tile.TileContext(nc) as tc:
        with tc.tile_pool(name="p", bufs=1) as p:
            t = p.tile([1, 128], BF16)
            nc.gpsimd.memset(t[:], 0.0)
            nc.sync.dma_start(out_d.ap()[0:1, 0, :], t[:])
    nc.compile()
    return nc


def null_time(in_maps, n=3):
    nc = build_null(T)
    bass_utils.run_bass_kernel_spmd(nc, in_maps, core_ids=list(range(NCORES)))
    times = []
    for _ in range(n):
        t0 = time.perf_counter()
        bass_utils.run_bass_kernel_spmd(nc, in_maps,
                                        core_ids=list(range(NCORES)))
        times.append(time.perf_counter() - t0)
    return min(times)


def kernel(**inputs):
    key = T
    if key not in _CACHE:
        _CACHE[key] = build(T)
    nc = _CACHE[key]
    in_maps = prepack(inputs, T)
    res = bass_utils.run_bass_kernel_spmd(nc, in_maps,
                                          core_ids=list(range(NCORES)))
    out = np.empty((T, B, F), np.float32)
    for c in range(NCORES):
        arr = res.results[c]["preds"]  # [128, T//4, 128]
        # row 32*tau+b of block blk -> preds[4*blk+tau, b, :]
        a = arr.reshape(4, 32, T // 4, 128).transpose(2, 0, 1, 3)
        out[:, BL * c:BL * (c + 1), :] = a.reshape(T, 32, 128)
    return out
